# revision 2
# baseline (speedup 1.0000x reference)
"""Trainium2 Bass kernel for DCTEncoderLayer.

Computes, for rgb_images_batch [32, 3, 512, 512] f32:
  ycbcr' = 2*rgb_to_ycbcr(rgb) - 1                 (per-pixel 3x3 channel mix, affine)
  32x32 block DCT per channel, coefficients scaled by (2/32)*c_u*c_v,
  output [32, 3*1024, 16, 16] with the frequency axis sorted by |(v,u)|.

Strategy (pure data parallel over batch, 4 images per NeuronCore):
  The 2D DCT is separable: coeff = Cs @ block @ Cs.T with Cs[v,y] =
  cos((2y+1)v*pi/64) * c_v / 4.  The YCbCr channel mix is a constant 3x3
  linear map, applied on the host (centering the input makes the affine
  offset vanish exactly); the device then runs a pure per-channel 2D DCT,
  which lets every tile use the full 128 partitions: 4 independent
  (image, block-row, channel) "units" x 32 rows.

  Per [128, 1024] tile (2 groups of 4 units):
    stage1:  t1p  = W.T @ x          (PE, fp8e3 moving, N=512 x2)
    cast:    t1sb = f16(t1p)         (ACT engine, PSUM -> SBUF)
    xpose:   tT   = T(t1sb) chunks   (DMA XBAR transpose, [128,128] x8,
                                      runs on SP/ACT DMA queues - no DVE)
    stage2:  o2p  = W.T @ tT         (PE, f16 moving, N=512 x2)
    cast:    osb  = f16(o2p)         (DVE engine, PSUM -> SBUF)
  Both stages use the SAME stationary weight W = blockdiag4(Cs.T) [128,128]
  f16.  Input ships as fp8 e3m4 (best-rounding fp8; halves input DMA),
  output ships as f16 (halves output DMA).  The host reassembles the raw
  [128, 24576] f16 device output and applies the frequency sort.
"""

import os
import sys

try:
    import concourse.bass  # noqa: F401
except ImportError:  # bare interpreter without the axon site paths
    sys.path.insert(0, "/opt/trn_rl_repo")

import numpy as np
import ml_dtypes

import concourse.bacc as bacc
import concourse.bass as bass
import concourse.mybir as mybir
import concourse.tile as tile
from concourse.bass_utils import run_bass_kernel_spmd

F32 = mybir.dt.float32
F16 = mybir.dt.float16
F8E3 = mybir.dt.float8e3

BS = 32            # DCT block size
N_CORES = 8
B_PER_CORE = 4     # batch images per core
NH = 16            # blocks per row/column (512/32)
UNITS = B_PER_CORE * NH * 3   # 192 (image, block-row, channel) units per core
GROUPS = UNITS // 4           # 48 groups of 4 units -> [128, 512] tiles
ITERS = GROUPS // 2           # 24 iterations of [128, 1024]

_STATE = {}
LAST_RESULT = None  # BassKernelResults of the most recent run (for profiling)


def _dct_mat():
    """Cs[v, y] = cos((2y+1) v pi / 64) * c_v / 4  (f64)."""
    y = np.arange(BS)
    v = np.arange(BS)[:, None]
    c = np.cos((2 * y + 1) * v * np.pi / (2 * BS))
    c[0, :] *= 1.0 / np.sqrt(2.0)
    return c / 4.0


def _sort_idx():
    # must replicate the reference's argsort (default kind) exactly,
    # including its tie order for equal |(v,u)|
    mag = np.zeros((BS, BS), dtype=np.float64)
    for v in range(BS):
        for u in range(BS):
            mag[v, u] = np.linalg.norm(np.array([v, u], dtype=np.int64))
    return np.argsort(mag.reshape(-1))


def _constants():
    cs = _dct_mat()
    w = np.zeros((128, 128))
    for s in range(4):
        w[s * 32 : (s + 1) * 32, s * 32 : (s + 1) * 32] = cs.T
    # rows (y', cb', cr') of the linear part of 2*rgb_to_ycbcr(rgb)-1, in (r,g,b)
    a2 = np.array(
        [
            [2 * 0.299, 2 * 0.587, 2 * 0.114],
            [2 * 0.564 * -0.299, 2 * 0.564 * -0.587, 2 * 0.564 * (1 - 0.114)],
            [2 * 0.713 * (1 - 0.299), 2 * 0.713 * -0.587, 2 * 0.713 * -0.114],
        ],
        np.float64,
    )
    return w.astype(np.float16), a2.astype(np.float32)


def _build_program():
    nc = bacc.Bacc(trn_type="TRN2")
    x = nc.dram_tensor("x", [128, GROUPS * 512], F8E3, kind="ExternalInput")
    w = nc.dram_tensor("w", [128, 128], F16, kind="ExternalInput")
    out = nc.dram_tensor("out", [128, GROUPS * 512], F16, kind="ExternalOutput")

    with tile.TileContext(nc) as tc:
        with (
            tc.tile_pool(name="const", bufs=1) as constp,
            tc.tile_pool(name="xin", bufs=3) as xinp,
            tc.tile_pool(name="t1s", bufs=2) as t1sp,
            tc.tile_pool(name="tts", bufs=2) as ttsp,
            tc.tile_pool(name="osb", bufs=2) as osbp,
            tc.tile_pool(name="psA", bufs=2, space="PSUM") as psA,
            tc.tile_pool(name="psB", bufs=2, space="PSUM") as psB,
        ):
            ws = constp.tile([128, 128], F16)
            nc.sync.dma_start(ws[:], w[:])

            osb = None
            for i in range(ITERS):
                if i % 2 == 0:
                    xin = xinp.tile([128, 2048], F8E3, tag="xin")
                    nc.sync.dma_start(
                        xin[:], x[:, (i // 2) * 2048 : (i // 2 + 1) * 2048]
                    )
                off = (i % 2) * 1024
                # stage 1: per-unit DCT over y (fp8e3 moving x f16 stationary)
                t1p = psA.tile([128, 1024], F32, tag="t1p")
                for h in range(2):
                    nc.tensor.matmul(
                        t1p[:, h * 512 : (h + 1) * 512],
                        ws[:],
                        xin[:, off + h * 512 : off + (h + 1) * 512],
                        start=True,
                        stop=True,
                    )
                t1sb = t1sp.tile([128, 1024], F16, tag="t1sb")
                nc.scalar.copy(t1sb[:], t1p[:])
                # XBAR transpose of each [128,128] chunk on the DMA queues
                tT = ttsp.tile([128, 1024], F16, tag="tT")
                for c in range(8):
                    eng = nc.sync if c < 6 else nc.scalar
                    eng.dma_start(
                        tT[:, c * 128 : (c + 1) * 128],
                        t1sb[:, c * 128 : (c + 1) * 128],
                        transpose=True,
                    )
                # stage 2: per-unit DCT over x (same stationary weight)
                o2p = psB.tile([128, 1024], F32, tag="o2p")
                for h in range(2):
                    nc.tensor.matmul(
                        o2p[:, h * 512 : (h + 1) * 512],
                        ws[:],
                        tT[:, h * 512 : (h + 1) * 512],
                        start=True,
                        stop=True,
                    )
                if i % 2 == 0:
                    osb = osbp.tile([128, 2048], F16, tag="osb")
                nc.vector.tensor_copy(osb[:, off : off + 1024], o2p[:])
                if i % 2 == 1:
                    nc.scalar.dma_start(
                        out[:, (i // 2) * 2048 : (i // 2 + 1) * 2048], osb[:]
                    )

    nc.finalize()
    return nc


def _get_program():
    if "nc" not in _STATE:
        _STATE["nc"] = _build_program()
        _STATE["consts"] = _constants()
        _STATE["sort_idx"] = _sort_idx()
    return _STATE["nc"]


def kernel(**inputs):
    global LAST_RESULT
    rgb = np.asarray(inputs["rgb_images_batch"], np.float32)
    assert rgb.shape == (N_CORES * B_PER_CORE, 3, 512, 512)
    nc = _get_program()
    w, a2 = _STATE["consts"]
    sort_idx = _STATE["sort_idx"]

    # centered YCbCr mix on host: offsets vanish exactly for centered input
    yc = np.tensordot(a2, rgb - np.float32(0.5), axes=([1], [1]))  # [3,32,512,512]
    yc = yc.transpose(1, 0, 2, 3)  # [32, 3, 512, 512]
    # per-core unit layout: n = b*48 + r*3 + c; group g = n//4, slot s = n%4
    xs = yc.reshape(N_CORES, B_PER_CORE, 3, NH, 32, 512)
    xs = xs.transpose(0, 1, 3, 2, 4, 5)  # [core, b, r, c, y, x]
    xs = np.ascontiguousarray(xs).reshape(N_CORES, GROUPS, 4, 32, 512)
    xs = xs.transpose(0, 2, 3, 1, 4)  # [core, s, y, g, x]
    xs = np.ascontiguousarray(xs).reshape(N_CORES, 128, GROUPS * 512)
    xs = xs.astype(ml_dtypes.float8_e3m4)

    in_maps = [{"x": xs[c], "w": w} for c in range(N_CORES)]
    trace = os.environ.get("KERNEL_TRACE", "0") == "1"
    res = run_bass_kernel_spmd(
        nc, in_maps, core_ids=list(range(N_CORES)), trace=trace
    )
    LAST_RESULT = res

    outs = []
    for c in range(N_CORES):
        dev = np.asarray(res.results[c]["out"])  # [128, 24576] f16
        # partition p = (g2, uf); column = (i, half, c2, s, v)
        a = dev.reshape(4, 32, ITERS, 2, 4, 4, 32)
        a = a.transpose(2, 3, 5, 6, 1, 4, 0)  # [i, half, s, v, uf, c2, g2]
        a = np.ascontiguousarray(a).reshape(UNITS, 32, 32, 16)  # [n, v, u, gx]
        a = a.reshape(B_PER_CORE, NH, 3, 32, 32, NH)  # [b, r, c, v, u, gx]
        a = a.transpose(0, 2, 3, 4, 1, 5)  # [b, c, v, u, r, gx]
        a = np.ascontiguousarray(a).reshape(B_PER_CORE, 3, 1024, NH, NH)
        a = a[:, :, sort_idx, :, :]
        outs.append(a.reshape(B_PER_CORE, 3 * 1024, NH, NH))
    return np.concatenate(outs, axis=0).astype(np.float32)


# revision 5
# speedup vs baseline: 4.9435x; 4.9435x over previous
"""Trainium2 Bass kernel for DCTEncoderLayer.

Computes, for rgb_images_batch [32, 3, 512, 512] f32:
  ycbcr' = 2*rgb_to_ycbcr(rgb) - 1                 (per-pixel 3x3 channel mix, affine)
  32x32 block DCT per channel, coefficients scaled by (2/32)*c_u*c_v,
  output [32, 3*1024, 16, 16] with the frequency axis sorted by |(v,u)|.

Strategy (pure data parallel over batch, 4 images per NeuronCore):
  The 2D DCT is separable: coeff = Cs @ block @ Cs.T with Cs[v,y] =
  cos((2y+1)v*pi/64) * c_v / 4.  The YCbCr channel mix is a constant 3x3
  linear map, applied on the host (centering the input makes the affine
  offset vanish exactly); the device then runs a pure per-channel 2D DCT,
  which lets every tile use the full 128 partitions: 4 independent
  (image, block-row, channel) "units" x 32 rows.

  Per [128, 1024] tile (2 groups of 4 units):
    stage1:  t1p[c] = X_chunk[c].T @ W   (PE, x8 chunks of 128 columns,
             DATA as the stationary operand - the result comes out of the
             PE already transposed (x on partitions), so no stream
             transpose / xbar transpose pass exists at all)
    cast:    t1sb = f16(t1p)             (ACT engine, PSUM -> SBUF)
    stage2:  o2p  = W.T @ t1sb           (PE, f16 moving, N=512 x2)
    cast:    osb  = f16(o2p)             (DVE engine, PSUM -> SBUF)
  Both stages use the SAME weight W = blockdiag4(Cs.T) [128,128] f16
  (moving in stage 1, stationary in stage 2).  Input ships as fp8 e3m4
  (best-rounding fp8; halves input DMA), output ships as f16 (halves
  output DMA).  The host reassembles the raw [128, 24576] f16 device
  output and applies the frequency sort.
"""

import os
import sys

try:
    import concourse.bass  # noqa: F401
except ImportError:  # bare interpreter without the axon site paths
    sys.path.insert(0, "/opt/trn_rl_repo")

import numpy as np
import ml_dtypes

import concourse.bacc as bacc
import concourse.bass as bass
import concourse.mybir as mybir
import concourse.tile as tile
from concourse.bass_utils import run_bass_kernel_spmd

F32 = mybir.dt.float32
F16 = mybir.dt.float16
F8E3 = mybir.dt.float8e3

BS = 32            # DCT block size
N_CORES = 8
B_PER_CORE = 4     # batch images per core
NH = 16            # blocks per row/column (512/32)
UNITS = B_PER_CORE * NH * 3   # 192 (image, block-row, channel) units per core
GROUPS = UNITS // 4           # 48 groups of 4 units -> [128, 512] tiles
ITERS = GROUPS // 2           # 24 iterations of [128, 1024]

_STATE = {}
LAST_RESULT = None  # BassKernelResults of the most recent run (for profiling)


def _dct_mat():
    """Cs[v, y] = cos((2y+1) v pi / 64) * c_v / 4  (f64)."""
    y = np.arange(BS)
    v = np.arange(BS)[:, None]
    c = np.cos((2 * y + 1) * v * np.pi / (2 * BS))
    c[0, :] *= 1.0 / np.sqrt(2.0)
    return c / 4.0


def _sort_idx():
    # must replicate the reference's argsort (default kind) exactly,
    # including its tie order for equal |(v,u)|
    mag = np.zeros((BS, BS), dtype=np.float64)
    for v in range(BS):
        for u in range(BS):
            mag[v, u] = np.linalg.norm(np.array([v, u], dtype=np.int64))
    return np.argsort(mag.reshape(-1))


def _constants():
    cs = _dct_mat()
    w = np.zeros((128, 128))
    for s in range(4):
        w[s * 32 : (s + 1) * 32, s * 32 : (s + 1) * 32] = cs.T
    # rows (y', cb', cr') of the linear part of 2*rgb_to_ycbcr(rgb)-1, in (r,g,b)
    a2 = np.array(
        [
            [2 * 0.299, 2 * 0.587, 2 * 0.114],
            [2 * 0.564 * -0.299, 2 * 0.564 * -0.587, 2 * 0.564 * (1 - 0.114)],
            [2 * 0.713 * (1 - 0.299), 2 * 0.713 * -0.587, 2 * 0.713 * -0.114],
        ],
        np.float64,
    )
    return w.astype(np.float16), a2.astype(np.float32)


def _build_program():
    nc = bacc.Bacc(trn_type="TRN2")
    x = nc.dram_tensor("x", [128, GROUPS * 512], F8E3, kind="ExternalInput")
    w = nc.dram_tensor("w", [128, 128], F16, kind="ExternalInput")
    out = nc.dram_tensor("out", [128, GROUPS * 512], F16, kind="ExternalOutput")

    with tile.TileContext(nc) as tc:
        with (
            tc.tile_pool(name="const", bufs=1) as constp,
            tc.tile_pool(name="xin", bufs=3) as xinp,
            tc.tile_pool(name="t1s", bufs=2) as t1sp,
            tc.tile_pool(name="osb", bufs=2) as osbp,
            tc.tile_pool(name="psA", bufs=2, space="PSUM") as psA,
            tc.tile_pool(name="psB", bufs=2, space="PSUM") as psB,
        ):
            ws = constp.tile([128, 128], F16)
            nc.sync.dma_start(ws[:], w[:])

            osb = None
            for i in range(ITERS):
                if i % 2 == 0:
                    xin = xinp.tile([128, 2048], F8E3, tag="xin")
                    nc.sync.dma_start(
                        xin[:], x[:, (i // 2) * 2048 : (i // 2 + 1) * 2048]
                    )
                off = (i % 2) * 1024
                # stage 1: per-unit DCT over y, data-stationary so the
                # result lands transposed (x on partitions)
                t1p = psA.tile([128, 1024], F32, tag="t1p")
                for c in range(8):
                    nc.tensor.matmul(
                        t1p[:, c * 128 : (c + 1) * 128],
                        xin[:, off + c * 128 : off + (c + 1) * 128],
                        ws[:],
                        start=True,
                        stop=True,
                    )
                t1sb = t1sp.tile([128, 1024], F16, tag="t1sb")
                nc.scalar.copy(t1sb[:], t1p[:])
                # stage 2: per-unit DCT over x (weight-stationary)
                o2p = psB.tile([128, 1024], F32, tag="o2p")
                for h in range(2):
                    nc.tensor.matmul(
                        o2p[:, h * 512 : (h + 1) * 512],
                        ws[:],
                        t1sb[:, h * 512 : (h + 1) * 512],
                        start=True,
                        stop=True,
                    )
                if i % 2 == 0:
                    osb = osbp.tile([128, 2048], F16, tag="osb")
                nc.vector.tensor_copy(osb[:, off : off + 1024], o2p[:])
                if i % 2 == 1:
                    nc.scalar.dma_start(
                        out[:, (i // 2) * 2048 : (i // 2 + 1) * 2048], osb[:]
                    )

    nc.finalize()
    return nc


def _get_program():
    if "nc" not in _STATE:
        _STATE["nc"] = _build_program()
        _STATE["consts"] = _constants()
        _STATE["sort_idx"] = _sort_idx()
    return _STATE["nc"]


def kernel(**inputs):
    global LAST_RESULT
    rgb = np.asarray(inputs["rgb_images_batch"], np.float32)
    assert rgb.shape == (N_CORES * B_PER_CORE, 3, 512, 512)
    nc = _get_program()
    w, a2 = _STATE["consts"]
    sort_idx = _STATE["sort_idx"]

    # centered YCbCr mix on host: offsets vanish exactly for centered input
    yc = np.tensordot(a2, rgb - np.float32(0.5), axes=([1], [1]))  # [3,32,512,512]
    yc = yc.transpose(1, 0, 2, 3)  # [32, 3, 512, 512]
    # per-core unit layout: n = b*48 + r*3 + c; group g = n//4, slot s = n%4
    xs = yc.reshape(N_CORES, B_PER_CORE, 3, NH, 32, 512)
    xs = xs.transpose(0, 1, 3, 2, 4, 5)  # [core, b, r, c, y, x]
    xs = np.ascontiguousarray(xs).reshape(N_CORES, GROUPS, 4, 32, 512)
    xs = xs.transpose(0, 2, 3, 1, 4)  # [core, s, y, g, x]
    xs = np.ascontiguousarray(xs).reshape(N_CORES, 128, GROUPS * 512)
    xs = xs.astype(ml_dtypes.float8_e3m4)

    in_maps = [{"x": xs[c], "w": w} for c in range(N_CORES)]
    trace = os.environ.get("KERNEL_TRACE", "0") == "1"
    res = run_bass_kernel_spmd(
        nc, in_maps, core_ids=list(range(N_CORES)), trace=trace
    )
    LAST_RESULT = res

    outs = []
    for c in range(N_CORES):
        dev = np.asarray(res.results[c]["out"])  # [128, 24576] f16
        # partition p = (g2, uf); column = (i, half, c2, s, v)
        a = dev.reshape(4, 32, ITERS, 2, 4, 4, 32)
        a = a.transpose(2, 3, 5, 6, 1, 4, 0)  # [i, half, s, v, uf, c2, g2]
        a = np.ascontiguousarray(a).reshape(UNITS, 32, 32, 16)  # [n, v, u, gx]
        a = a.reshape(B_PER_CORE, NH, 3, 32, 32, NH)  # [b, r, c, v, u, gx]
        a = a.transpose(0, 2, 3, 4, 1, 5)  # [b, c, v, u, r, gx]
        a = np.ascontiguousarray(a).reshape(B_PER_CORE, 3, 1024, NH, NH)
        a = a[:, :, sort_idx, :, :]
        outs.append(a.reshape(B_PER_CORE, 3 * 1024, NH, NH))
    return np.concatenate(outs, axis=0).astype(np.float32)


# revision 8
# speedup vs baseline: 5.0964x; 1.0309x over previous
"""Trainium2 Bass kernel for DCTEncoderLayer.

Computes, for rgb_images_batch [32, 3, 512, 512] f32:
  ycbcr' = 2*rgb_to_ycbcr(rgb) - 1                 (per-pixel 3x3 channel mix, affine)
  32x32 block DCT per channel, coefficients scaled by (2/32)*c_u*c_v,
  output [32, 3*1024, 16, 16] with the frequency axis sorted by |(v,u)|.

Strategy (pure data parallel over batch, 4 images per NeuronCore):
  The 2D DCT is separable: coeff = Cs @ block @ Cs.T with Cs[v,y] =
  cos((2y+1)v*pi/64) * c_v / 4.  The YCbCr channel mix is a constant 3x3
  linear map, applied on the host (centering the input makes the affine
  offset vanish exactly); the device then runs a pure per-channel 2D DCT,
  which lets every tile use the full 128 partitions: 4 independent
  (image, block-row, channel) "units" x 32 rows.

  Per [128, 1024] tile (2 groups of 4 units):
    stage1:  t1p[c] = X_chunk[c].T @ W   (PE, x8 chunks of 128 columns,
             DATA as the stationary operand - the result comes out of the
             PE already transposed (x on partitions), so no stream
             transpose / xbar transpose pass exists at all)
    cast:    t1sb = f16(t1p)             (ACT engine, PSUM -> SBUF)
    stage2:  o2p  = W.T @ t1sb           (PE, f16 moving, N=512 x2)
    cast:    osb  = f16(o2p)             (DVE engine, PSUM -> SBUF)
  Both stages use the SAME weight W = blockdiag4(Cs.T) [128,128] f16
  (moving in stage 1, stationary in stage 2).  Input ships as fp8 e3m4
  (best-rounding fp8; halves input DMA), output ships as f16 (halves
  output DMA).  The host reassembles the raw [128, 24576] f16 device
  output and applies the frequency sort.
"""

import os
import sys

try:
    import concourse.bass  # noqa: F401
except ImportError:  # bare interpreter without the axon site paths
    sys.path.insert(0, "/opt/trn_rl_repo")

import numpy as np
import ml_dtypes

import concourse.bacc as bacc
import concourse.bass as bass
import concourse.mybir as mybir
import concourse.tile as tile
from concourse.bass_utils import run_bass_kernel_spmd

F32 = mybir.dt.float32
F16 = mybir.dt.float16
F8E3 = mybir.dt.float8e3

BS = 32            # DCT block size
N_CORES = 8
B_PER_CORE = 4     # batch images per core
NH = 16            # blocks per row/column (512/32)
UNITS = B_PER_CORE * NH * 3   # 192 (image, block-row, channel) units per core
GROUPS = UNITS // 4           # 48 groups of 4 units -> [128, 512] tiles
ITERS = GROUPS // 2           # 24 iterations of [128, 1024]

_STATE = {}
LAST_RESULT = None  # BassKernelResults of the most recent run (for profiling)


def _dct_mat():
    """Cs[v, y] = cos((2y+1) v pi / 64) * c_v / 4  (f64)."""
    y = np.arange(BS)
    v = np.arange(BS)[:, None]
    c = np.cos((2 * y + 1) * v * np.pi / (2 * BS))
    c[0, :] *= 1.0 / np.sqrt(2.0)
    return c / 4.0


def _sort_idx():
    # must replicate the reference's argsort (default kind) exactly,
    # including its tie order for equal |(v,u)|
    mag = np.zeros((BS, BS), dtype=np.float64)
    for v in range(BS):
        for u in range(BS):
            mag[v, u] = np.linalg.norm(np.array([v, u], dtype=np.int64))
    return np.argsort(mag.reshape(-1))


def _constants():
    cs = _dct_mat()
    w = np.zeros((128, 128))
    for s in range(4):
        w[s * 32 : (s + 1) * 32, s * 32 : (s + 1) * 32] = cs.T
    # rows (y', cb', cr') of the linear part of 2*rgb_to_ycbcr(rgb)-1, in (r,g,b)
    a2 = np.array(
        [
            [2 * 0.299, 2 * 0.587, 2 * 0.114],
            [2 * 0.564 * -0.299, 2 * 0.564 * -0.587, 2 * 0.564 * (1 - 0.114)],
            [2 * 0.713 * (1 - 0.299), 2 * 0.713 * -0.587, 2 * 0.713 * -0.114],
        ],
        np.float64,
    )
    return w.astype(np.float16), a2.astype(np.float32)


def _build_program():
    nc = bacc.Bacc(trn_type="TRN2")
    x = nc.dram_tensor("x", [128, GROUPS * 512], F8E3, kind="ExternalInput")
    w = nc.dram_tensor("w", [128, 128], F16, kind="ExternalInput")
    out = nc.dram_tensor("out", [128, GROUPS * 512], F16, kind="ExternalOutput")

    with tile.TileContext(nc) as tc:
        with (
            tc.tile_pool(name="const", bufs=1) as constp,
            tc.tile_pool(name="xin", bufs=3) as xinp,
            tc.tile_pool(name="t1s", bufs=2) as t1sp,
            tc.tile_pool(name="osb", bufs=2) as osbp,
            tc.tile_pool(name="psA", bufs=2, space="PSUM") as psA,
            tc.tile_pool(name="psB", bufs=2, space="PSUM") as psB,
        ):
            ws = constp.tile([128, 128], F16)
            nc.sync.dma_start(ws[:], w[:])

            # Software-pipelined: loop step i emits MM1(i) before MM2(i-1)
            # so the PE queue (strict FIFO) always has runnable work while
            # the ACT cast of iteration i is still in flight.
            osb = {}
            xin = None
            t1sb = {}

            def stage2(j):
                # stage 2 for iteration j: per-unit DCT over x
                o2p = psB.tile([128, 1024], F32, tag="o2p")
                for h in range(2):
                    nc.tensor.matmul(
                        o2p[:, h * 512 : (h + 1) * 512],
                        ws[:],
                        t1sb[j][:, h * 512 : (h + 1) * 512],
                        start=True,
                        stop=True,
                    )
                if j % 2 == 0:
                    osb[j // 2] = osbp.tile([128, 2048], F16, tag="osb", name=f"osb{j//2}")
                nc.vector.tensor_copy(
                    osb[j // 2][:, (j % 2) * 1024 : (j % 2 + 1) * 1024], o2p[:]
                )
                if j % 2 == 1:
                    nc.scalar.dma_start(
                        out[:, (j // 2) * 2048 : (j // 2 + 1) * 2048],
                        osb[j // 2][:],
                    )

            for i in range(ITERS):
                if i % 2 == 0:
                    xin = xinp.tile([128, 2048], F8E3, tag="xin")
                    nc.sync.dma_start(
                        xin[:], x[:, (i // 2) * 2048 : (i // 2 + 1) * 2048]
                    )
                off = (i % 2) * 1024
                # stage 1: per-unit DCT over y, data-stationary so the
                # result lands transposed (x on partitions)
                t1p = psA.tile([128, 1024], F32, tag="t1p")
                for c in range(8):
                    nc.tensor.matmul(
                        t1p[:, c * 128 : (c + 1) * 128],
                        xin[:, off + c * 128 : off + (c + 1) * 128],
                        ws[:],
                        start=True,
                        stop=True,
                    )
                if i > 0:
                    stage2(i - 1)
                t1sb[i] = t1sp.tile([128, 1024], F16, tag="t1sb", name=f"t1sb{i}")
                nc.scalar.copy(t1sb[i][:], t1p[:])
                t1sb.pop(i - 2, None)
            stage2(ITERS - 1)

    nc.finalize()
    return nc


def _get_program():
    if "nc" not in _STATE:
        _STATE["nc"] = _build_program()
        _STATE["consts"] = _constants()
        _STATE["sort_idx"] = _sort_idx()
    return _STATE["nc"]


def kernel(**inputs):
    global LAST_RESULT
    rgb = np.asarray(inputs["rgb_images_batch"], np.float32)
    assert rgb.shape == (N_CORES * B_PER_CORE, 3, 512, 512)
    nc = _get_program()
    w, a2 = _STATE["consts"]
    sort_idx = _STATE["sort_idx"]

    # centered YCbCr mix on host: offsets vanish exactly for centered input
    yc = np.tensordot(a2, rgb - np.float32(0.5), axes=([1], [1]))  # [3,32,512,512]
    yc = yc.transpose(1, 0, 2, 3)  # [32, 3, 512, 512]
    # per-core unit layout: n = b*48 + r*3 + c; group g = n//4, slot s = n%4
    xs = yc.reshape(N_CORES, B_PER_CORE, 3, NH, 32, 512)
    xs = xs.transpose(0, 1, 3, 2, 4, 5)  # [core, b, r, c, y, x]
    xs = np.ascontiguousarray(xs).reshape(N_CORES, GROUPS, 4, 32, 512)
    xs = xs.transpose(0, 2, 3, 1, 4)  # [core, s, y, g, x]
    xs = np.ascontiguousarray(xs).reshape(N_CORES, 128, GROUPS * 512)
    xs = xs.astype(ml_dtypes.float8_e3m4)

    in_maps = [{"x": xs[c], "w": w} for c in range(N_CORES)]
    trace = os.environ.get("KERNEL_TRACE", "0") == "1"
    res = run_bass_kernel_spmd(
        nc, in_maps, core_ids=list(range(N_CORES)), trace=trace
    )
    LAST_RESULT = res

    outs = []
    for c in range(N_CORES):
        dev = np.asarray(res.results[c]["out"])  # [128, 24576] f16
        # partition p = (g2, uf); column = (i, half, c2, s, v)
        a = dev.reshape(4, 32, ITERS, 2, 4, 4, 32)
        a = a.transpose(2, 3, 5, 6, 1, 4, 0)  # [i, half, s, v, uf, c2, g2]
        a = np.ascontiguousarray(a).reshape(UNITS, 32, 32, 16)  # [n, v, u, gx]
        a = a.reshape(B_PER_CORE, NH, 3, 32, 32, NH)  # [b, r, c, v, u, gx]
        a = a.transpose(0, 2, 3, 4, 1, 5)  # [b, c, v, u, r, gx]
        a = np.ascontiguousarray(a).reshape(B_PER_CORE, 3, 1024, NH, NH)
        a = a[:, :, sort_idx, :, :]
        outs.append(a.reshape(B_PER_CORE, 3 * 1024, NH, NH))
    return np.concatenate(outs, axis=0).astype(np.float32)


# revision 10
# speedup vs baseline: 5.8581x; 1.1495x over previous
"""Trainium2 Bass kernel for DCTEncoderLayer.

Computes, for rgb_images_batch [32, 3, 512, 512] f32:
  ycbcr' = 2*rgb_to_ycbcr(rgb) - 1                 (per-pixel 3x3 channel mix, affine)
  32x32 block DCT per channel, coefficients scaled by (2/32)*c_u*c_v,
  output [32, 3*1024, 16, 16] with the frequency axis sorted by |(v,u)|.

Strategy (pure data parallel over batch, 4 images per NeuronCore):
  The 2D DCT is separable: coeff = Cs @ block @ Cs.T with Cs[v,y] =
  cos((2y+1)v*pi/64) * c_v / 4.  The YCbCr channel mix is a constant 3x3
  linear map, applied on the host (centering the input makes the affine
  offset vanish exactly); the device then runs a pure per-channel 2D DCT,
  which lets every tile use the full 128 partitions: 4 independent
  (image, block-row, channel) "units" x 32 rows.

  Per [128, 1024] tile (2 groups of 4 units):
    stage1:  t1p[c] = X_chunk[c].T @ W   (PE, x8 chunks of 128 columns,
             DATA as the stationary operand - the result comes out of the
             PE already transposed (x on partitions), so no stream
             transpose / xbar transpose pass exists at all)
    cast:    t1sb = f16(t1p)             (ACT engine, PSUM -> SBUF)
    stage2:  o2p  = W.T @ t1sb           (PE, f16 moving, N=512 x2)
    cast:    osb  = f16(o2p)             (DVE engine, PSUM -> SBUF)
  Both stages use the SAME weight W = blockdiag4(Cs.T) [128,128] f16
  (moving in stage 1, stationary in stage 2).  Input ships as fp8 e3m4
  (best-rounding fp8; halves input DMA), output ships as f16 (halves
  output DMA).  The host reassembles the raw [128, 24576] f16 device
  output and applies the frequency sort.
"""

import os
import sys

try:
    import concourse.bass  # noqa: F401
except ImportError:  # bare interpreter without the axon site paths
    sys.path.insert(0, "/opt/trn_rl_repo")

import numpy as np
import ml_dtypes

import concourse.bacc as bacc
import concourse.bass as bass
import concourse.mybir as mybir
import concourse.tile as tile
from concourse.bass_utils import run_bass_kernel_spmd

F32 = mybir.dt.float32
F16 = mybir.dt.float16
F8E3 = mybir.dt.float8e3

BS = 32            # DCT block size
N_CORES = 8
B_PER_CORE = 4     # batch images per core
NH = 16            # blocks per row/column (512/32)
UNITS = B_PER_CORE * NH * 3   # 192 (image, block-row, channel) units per core
GROUPS = UNITS // 4           # 48 groups of 4 units -> [128, 512] tiles
ITERS = GROUPS // 2           # 24 iterations of [128, 1024]

_STATE = {}
LAST_RESULT = None  # BassKernelResults of the most recent run (for profiling)


def _dct_mat():
    """Cs[v, y] = cos((2y+1) v pi / 64) * c_v / 4  (f64)."""
    y = np.arange(BS)
    v = np.arange(BS)[:, None]
    c = np.cos((2 * y + 1) * v * np.pi / (2 * BS))
    c[0, :] *= 1.0 / np.sqrt(2.0)
    return c / 4.0


def _sort_idx():
    # must replicate the reference's argsort (default kind) exactly,
    # including its tie order for equal |(v,u)|
    mag = np.zeros((BS, BS), dtype=np.float64)
    for v in range(BS):
        for u in range(BS):
            mag[v, u] = np.linalg.norm(np.array([v, u], dtype=np.int64))
    return np.argsort(mag.reshape(-1))


def _constants():
    cs = _dct_mat()
    w = np.zeros((128, 128))
    for s in range(4):
        w[s * 32 : (s + 1) * 32, s * 32 : (s + 1) * 32] = cs.T
    # rows (y', cb', cr') of the linear part of 2*rgb_to_ycbcr(rgb)-1, in (r,g,b)
    a2 = np.array(
        [
            [2 * 0.299, 2 * 0.587, 2 * 0.114],
            [2 * 0.564 * -0.299, 2 * 0.564 * -0.587, 2 * 0.564 * (1 - 0.114)],
            [2 * 0.713 * (1 - 0.299), 2 * 0.713 * -0.587, 2 * 0.713 * -0.114],
        ],
        np.float64,
    )
    return w.astype(np.float16), a2.astype(np.float32)


def _build_program():
    nc = bacc.Bacc(trn_type="TRN2")
    x = nc.dram_tensor("x", [128, GROUPS * 512], F8E3, kind="ExternalInput")
    w = nc.dram_tensor("w", [128, 128], F16, kind="ExternalInput")
    out = nc.dram_tensor("out", [128, GROUPS * 512], F16, kind="ExternalOutput")

    with tile.TileContext(nc) as tc:
        with (
            tc.tile_pool(name="const", bufs=1) as constp,
            tc.tile_pool(name="xin", bufs=3) as xinp,
            tc.tile_pool(name="t1s", bufs=3) as t1sp,
            tc.tile_pool(name="osb", bufs=3) as osbp,
            tc.tile_pool(name="psA", bufs=2, space="PSUM") as psA,
            tc.tile_pool(name="psB", bufs=2, space="PSUM") as psB,
        ):
            ws = constp.tile([128, 128], F16)
            nc.sync.dma_start(ws[:], w[:])

            # Software-pipelined: loop step i emits MM1(i) before MM2(i-1)
            # so the PE queue (strict FIFO) always has runnable work while
            # the ACT cast of iteration i is still in flight.
            osb = {}
            xin = None
            t1sb = {}

            def stage2(j):
                # stage 2 for iteration j: per-unit DCT over x
                o2p = psB.tile([128, 1024], F32, tag="o2p")
                for h in range(2):
                    nc.tensor.matmul(
                        o2p[:, h * 512 : (h + 1) * 512],
                        ws[:],
                        t1sb[j][:, h * 512 : (h + 1) * 512],
                        start=True,
                        stop=True,
                    )
                if j % 2 == 0:
                    osb[j // 2] = osbp.tile([128, 2048], F16, tag="osb", name=f"osb{j//2}")
                nc.vector.tensor_copy(
                    osb[j // 2][:, (j % 2) * 1024 : (j % 2 + 1) * 1024], o2p[:]
                )
                if j % 2 == 1:
                    nc.sync.dma_start(
                        out[:, (j // 2) * 2048 : (j // 2 + 1) * 2048],
                        osb[j // 2][:],
                    )

            for i in range(ITERS):
                if i % 2 == 0:
                    xin = xinp.tile([128, 2048], F8E3, tag="xin")
                    nc.sync.dma_start(
                        xin[:], x[:, (i // 2) * 2048 : (i // 2 + 1) * 2048]
                    )
                off = (i % 2) * 1024
                # stage 1: per-unit DCT over y, data-stationary so the
                # result lands transposed (x on partitions)
                t1p = psA.tile([128, 1024], F32, tag="t1p")
                for c in range(8):
                    nc.tensor.matmul(
                        t1p[:, c * 128 : (c + 1) * 128],
                        xin[:, off + c * 128 : off + (c + 1) * 128],
                        ws[:],
                        start=True,
                        stop=True,
                    )
                if i > 0:
                    stage2(i - 1)
                t1sb[i] = t1sp.tile([128, 1024], F16, tag="t1sb", name=f"t1sb{i}")
                nc.scalar.copy(t1sb[i][:], t1p[:])
                t1sb.pop(i - 2, None)
            stage2(ITERS - 1)

    nc.finalize()
    return nc


def _get_program():
    if "nc" not in _STATE:
        _STATE["nc"] = _build_program()
        _STATE["consts"] = _constants()
        _STATE["sort_idx"] = _sort_idx()
    return _STATE["nc"]


def kernel(**inputs):
    global LAST_RESULT
    rgb = np.asarray(inputs["rgb_images_batch"], np.float32)
    assert rgb.shape == (N_CORES * B_PER_CORE, 3, 512, 512)
    nc = _get_program()
    w, a2 = _STATE["consts"]
    sort_idx = _STATE["sort_idx"]

    # centered YCbCr mix on host: offsets vanish exactly for centered input
    yc = np.tensordot(a2, rgb - np.float32(0.5), axes=([1], [1]))  # [3,32,512,512]
    yc = yc.transpose(1, 0, 2, 3)  # [32, 3, 512, 512]
    # per-core unit layout: n = b*48 + r*3 + c; group g = n//4, slot s = n%4
    xs = yc.reshape(N_CORES, B_PER_CORE, 3, NH, 32, 512)
    xs = xs.transpose(0, 1, 3, 2, 4, 5)  # [core, b, r, c, y, x]
    xs = np.ascontiguousarray(xs).reshape(N_CORES, GROUPS, 4, 32, 512)
    xs = xs.transpose(0, 2, 3, 1, 4)  # [core, s, y, g, x]
    xs = np.ascontiguousarray(xs).reshape(N_CORES, 128, GROUPS * 512)
    xs = xs.astype(ml_dtypes.float8_e3m4)

    in_maps = [{"x": xs[c], "w": w} for c in range(N_CORES)]
    trace = os.environ.get("KERNEL_TRACE", "0") == "1"
    res = run_bass_kernel_spmd(
        nc, in_maps, core_ids=list(range(N_CORES)), trace=trace
    )
    LAST_RESULT = res

    outs = []
    for c in range(N_CORES):
        dev = np.asarray(res.results[c]["out"])  # [128, 24576] f16
        # partition p = (g2, uf); column = (i, half, c2, s, v)
        a = dev.reshape(4, 32, ITERS, 2, 4, 4, 32)
        a = a.transpose(2, 3, 5, 6, 1, 4, 0)  # [i, half, s, v, uf, c2, g2]
        a = np.ascontiguousarray(a).reshape(UNITS, 32, 32, 16)  # [n, v, u, gx]
        a = a.reshape(B_PER_CORE, NH, 3, 32, 32, NH)  # [b, r, c, v, u, gx]
        a = a.transpose(0, 2, 3, 4, 1, 5)  # [b, c, v, u, r, gx]
        a = np.ascontiguousarray(a).reshape(B_PER_CORE, 3, 1024, NH, NH)
        a = a[:, :, sort_idx, :, :]
        outs.append(a.reshape(B_PER_CORE, 3 * 1024, NH, NH))
    return np.concatenate(outs, axis=0).astype(np.float32)


# revision 13
# speedup vs baseline: 6.0628x; 1.0349x over previous
"""Trainium2 Bass kernel for DCTEncoderLayer.

Computes, for rgb_images_batch [32, 3, 512, 512] f32:
  ycbcr' = 2*rgb_to_ycbcr(rgb) - 1                 (per-pixel 3x3 channel mix, affine)
  32x32 block DCT per channel, coefficients scaled by (2/32)*c_u*c_v,
  output [32, 3*1024, 16, 16] with the frequency axis sorted by |(v,u)|.

Strategy (pure data parallel over batch, 4 images per NeuronCore):
  The 2D DCT is separable: coeff = Cs @ block @ Cs.T with Cs[v,y] =
  cos((2y+1)v*pi/64) * c_v / 4.  The YCbCr channel mix is a constant 3x3
  linear map, applied on the host (centering the input makes the affine
  offset vanish exactly); the device then runs a pure per-channel 2D DCT,
  which lets every tile use the full 128 partitions: 4 independent
  (image, block-row, channel) "units" x 32 rows.

  Per [128, 1024] tile (2 groups of 4 units):
    stage1:  t1p[c] = X_chunk[c].T @ W   (PE, x8 chunks of 128 columns,
             DATA as the stationary operand - the result comes out of the
             PE already transposed (x on partitions), so no stream
             transpose / xbar transpose pass exists at all)
    cast:    t1sb = f16(t1p)             (ACT engine, PSUM -> SBUF)
    stage2:  o2p  = W.T @ t1sb           (PE, f16 moving, N=512 x2)
    cast:    osb  = f16(o2p)             (DVE engine, PSUM -> SBUF)
  Both stages use the SAME weight W = blockdiag4(Cs.T) [128,128] f16
  (moving in stage 1, stationary in stage 2).  Input ships as fp8 e3m4
  (best-rounding fp8; halves input DMA), output ships as f16 (halves
  output DMA).  The host reassembles the raw [128, 24576] f16 device
  output and applies the frequency sort.
"""

import os
import sys

try:
    import concourse.bass  # noqa: F401
except ImportError:  # bare interpreter without the axon site paths
    sys.path.insert(0, "/opt/trn_rl_repo")

import numpy as np
import ml_dtypes

import concourse.bacc as bacc
import concourse.bass as bass
import concourse.mybir as mybir
import concourse.tile as tile
from concourse.bass_utils import run_bass_kernel_spmd

F32 = mybir.dt.float32
F16 = mybir.dt.float16
F8E3 = mybir.dt.float8e3

BS = 32            # DCT block size
N_CORES = 8
B_PER_CORE = 4     # batch images per core
NH = 16            # blocks per row/column (512/32)
UNITS = B_PER_CORE * NH * 3   # 192 (image, block-row, channel) units per core
GROUPS = UNITS // 4           # 48 groups of 4 units -> [128, 512] tiles
ITERS = GROUPS // 2           # 24 iterations of [128, 1024]

_STATE = {}
LAST_RESULT = None  # BassKernelResults of the most recent run (for profiling)


def _dct_mat():
    """Cs[v, y] = cos((2y+1) v pi / 64) * c_v / 4  (f64)."""
    y = np.arange(BS)
    v = np.arange(BS)[:, None]
    c = np.cos((2 * y + 1) * v * np.pi / (2 * BS))
    c[0, :] *= 1.0 / np.sqrt(2.0)
    return c / 4.0


def _sort_idx():
    # must replicate the reference's argsort (default kind) exactly,
    # including its tie order for equal |(v,u)|
    mag = np.zeros((BS, BS), dtype=np.float64)
    for v in range(BS):
        for u in range(BS):
            mag[v, u] = np.linalg.norm(np.array([v, u], dtype=np.int64))
    return np.argsort(mag.reshape(-1))


def _constants():
    cs = _dct_mat()
    w = np.zeros((128, 128))
    for s in range(4):
        w[s * 32 : (s + 1) * 32, s * 32 : (s + 1) * 32] = cs.T
    # rows (y', cb', cr') of the linear part of 2*rgb_to_ycbcr(rgb)-1, in (r,g,b)
    a2 = np.array(
        [
            [2 * 0.299, 2 * 0.587, 2 * 0.114],
            [2 * 0.564 * -0.299, 2 * 0.564 * -0.587, 2 * 0.564 * (1 - 0.114)],
            [2 * 0.713 * (1 - 0.299), 2 * 0.713 * -0.587, 2 * 0.713 * -0.114],
        ],
        np.float64,
    )
    return w.astype(np.float16), a2.astype(np.float32)


def _build_program():
    nc = bacc.Bacc(trn_type="TRN2")
    x = nc.dram_tensor("x", [128, GROUPS * 512], F8E3, kind="ExternalInput")
    w = nc.dram_tensor("w", [128, 128], F16, kind="ExternalInput")
    out = nc.dram_tensor("out", [128, GROUPS * 512], F16, kind="ExternalOutput")

    with tile.TileContext(nc) as tc:
        with (
            tc.tile_pool(name="const", bufs=1) as constp,
            tc.tile_pool(name="xin", bufs=3) as xinp,
            tc.tile_pool(name="t1s", bufs=3) as t1sp,
            tc.tile_pool(name="osb", bufs=3) as osbp,
            tc.tile_pool(name="psA", bufs=4, space="PSUM") as psA,
        ):
            ws = constp.tile([128, 128], F16)
            nc.sync.dma_start(ws[:], w[:])

            # Software-pipelined: loop step i emits MM1(i) before MM2(i-1)
            # so the PE queue (strict FIFO) always has runnable work while
            # the ACT cast of iteration i is still in flight.
            osb = {}
            xin = None
            t1sb = {}

            t1ps = {}

            def stage2(j):
                # stage 2 for iteration j: per-unit DCT over x.  Writes into
                # the SAME PSUM tile stage 1 used (its contents are dead once
                # the ACT cast has read them) - one 8-bank pool, 4-deep.
                o2p = t1ps.pop(j)
                for h in range(2):
                    nc.tensor.matmul(
                        o2p[:, h * 512 : (h + 1) * 512],
                        ws[:],
                        t1sb[j][:, h * 512 : (h + 1) * 512],
                        start=True,
                        stop=True,
                    )
                if j % 2 == 0:
                    osb[j // 2] = osbp.tile([128, 2048], F16, tag="osb", name=f"osb{j//2}")
                nc.vector.tensor_copy(
                    osb[j // 2][:, (j % 2) * 1024 : (j % 2 + 1) * 1024], o2p[:]
                )
                if j % 2 == 1:
                    nc.sync.dma_start(
                        out[:, (j // 2) * 2048 : (j // 2 + 1) * 2048],
                        osb[j // 2][:],
                    )

            for i in range(ITERS):
                if i % 2 == 0:
                    xin = xinp.tile([128, 2048], F8E3, tag="xin")
                    nc.sync.dma_start(
                        xin[:], x[:, (i // 2) * 2048 : (i // 2 + 1) * 2048]
                    )
                off = (i % 2) * 1024
                # stage 1: per-unit DCT over y, data-stationary so the
                # result lands transposed (x on partitions)
                t1p = psA.tile([128, 1024], F32, tag="t1p", name=f"t1p{i}")
                t1ps[i] = t1p
                for c in range(8):
                    nc.tensor.matmul(
                        t1p[:, c * 128 : (c + 1) * 128],
                        xin[:, off + c * 128 : off + (c + 1) * 128],
                        ws[:],
                        start=True,
                        stop=True,
                    )
                if i > 0:
                    stage2(i - 1)
                t1sb[i] = t1sp.tile([128, 1024], F16, tag="t1sb", name=f"t1sb{i}")
                nc.scalar.copy(t1sb[i][:], t1p[:])
                t1sb.pop(i - 2, None)
            stage2(ITERS - 1)

    nc.finalize()
    return nc


def _get_program():
    if "nc" not in _STATE:
        _STATE["nc"] = _build_program()
        _STATE["consts"] = _constants()
        _STATE["sort_idx"] = _sort_idx()
    return _STATE["nc"]


def kernel(**inputs):
    global LAST_RESULT
    rgb = np.asarray(inputs["rgb_images_batch"], np.float32)
    assert rgb.shape == (N_CORES * B_PER_CORE, 3, 512, 512)
    nc = _get_program()
    w, a2 = _STATE["consts"]
    sort_idx = _STATE["sort_idx"]

    # centered YCbCr mix on host: offsets vanish exactly for centered input
    yc = np.tensordot(a2, rgb - np.float32(0.5), axes=([1], [1]))  # [3,32,512,512]
    yc = yc.transpose(1, 0, 2, 3)  # [32, 3, 512, 512]
    # per-core unit layout: n = b*48 + r*3 + c; group g = n//4, slot s = n%4
    xs = yc.reshape(N_CORES, B_PER_CORE, 3, NH, 32, 512)
    xs = xs.transpose(0, 1, 3, 2, 4, 5)  # [core, b, r, c, y, x]
    xs = np.ascontiguousarray(xs).reshape(N_CORES, GROUPS, 4, 32, 512)
    xs = xs.transpose(0, 2, 3, 1, 4)  # [core, s, y, g, x]
    xs = np.ascontiguousarray(xs).reshape(N_CORES, 128, GROUPS * 512)
    xs = xs.astype(ml_dtypes.float8_e3m4)

    in_maps = [{"x": xs[c], "w": w} for c in range(N_CORES)]
    trace = os.environ.get("KERNEL_TRACE", "0") == "1"
    res = run_bass_kernel_spmd(
        nc, in_maps, core_ids=list(range(N_CORES)), trace=trace
    )
    LAST_RESULT = res

    outs = []
    for c in range(N_CORES):
        dev = np.asarray(res.results[c]["out"])  # [128, 24576] f16
        # partition p = (g2, uf); column = (i, half, c2, s, v)
        a = dev.reshape(4, 32, ITERS, 2, 4, 4, 32)
        a = a.transpose(2, 3, 5, 6, 1, 4, 0)  # [i, half, s, v, uf, c2, g2]
        a = np.ascontiguousarray(a).reshape(UNITS, 32, 32, 16)  # [n, v, u, gx]
        a = a.reshape(B_PER_CORE, NH, 3, 32, 32, NH)  # [b, r, c, v, u, gx]
        a = a.transpose(0, 2, 3, 4, 1, 5)  # [b, c, v, u, r, gx]
        a = np.ascontiguousarray(a).reshape(B_PER_CORE, 3, 1024, NH, NH)
        a = a[:, :, sort_idx, :, :]
        outs.append(a.reshape(B_PER_CORE, 3 * 1024, NH, NH))
    return np.concatenate(outs, axis=0).astype(np.float32)
